# revision 17
# baseline (speedup 1.0000x reference)
"""BiDAF attention kernel for 8 Trainium2 NeuronCores.

Data-parallel over batch (B=32 -> 4 per core). Per batch, on-chip:
  sT[j,i] = (q*cqw) @ c^T + s0[i] + s1[j] + bias   (bf16 matmuls, fp32 accum;
  s0 comes free as row 64 of the same matmul, then a rank-1 augmentation
  matmul broadcasts it across rows)
  E = exp(sT)  (one exp serves both softmaxes; s1+bias fused via act bias)
  a2T = E / rowsum(E);  a1 normalization deferred: 1/colsum(E) applied to
  output rows of downstream matmuls.
  a = a1 @ q; binner = a1 @ a2^T; b = binner @ c; out = [c, a, c*a, c*b]
"""

import sys

if "/opt/trn_rl_repo" not in sys.path:
    sys.path.insert(0, "/opt/trn_rl_repo")

from contextlib import ExitStack

import numpy as np

import concourse.bacc as bacc
import concourse.bass as bass
import concourse.mybir as mybir
from concourse.bass import ts
from concourse.bass_utils import run_bass_kernel_spmd
from concourse.masks import make_identity
from concourse.tile import TileContext

N_CORES = 8
B, Lc, Lq, H = 32, 512, 64, 512
BPC = B // N_CORES  # batches per core
F32 = mybir.dt.float32
BF16 = mybir.dt.bfloat16
MULT = mybir.AluOpType.mult

_CACHE = {}


def _build_program():
    nc = bacc.Bacc("TRN2", target_bir_lowering=False, debug=False, num_devices=N_CORES)
    c_h = nc.dram_tensor("c", [BPC, Lc, H], F32, kind="ExternalInput")
    q_h = nc.dram_tensor("q", [BPC, Lq, H], F32, kind="ExternalInput")
    cqw_h = nc.dram_tensor("cqw", [H], F32, kind="ExternalInput")
    cwgt_h = nc.dram_tensor("cwgt", [H], F32, kind="ExternalInput")
    qwgt_h = nc.dram_tensor("qwgt", [H], F32, kind="ExternalInput")
    bias_h = nc.dram_tensor("bias", [1], F32, kind="ExternalInput")
    out_h = nc.dram_tensor("out", [BPC, Lc, 4 * H], F32, kind="ExternalOutput")

    c_ap = c_h.ap()
    q_ap = q_h.ap()
    out_ap = out_h.ap()

    exp_f = mybir.ActivationFunctionType.Exp
    ident_f = mybir.ActivationFunctionType.Identity
    copy_f = mybir.ActivationFunctionType.Copy

    with TileContext(nc) as tc, ExitStack() as ctx:
        const = ctx.enter_context(tc.tile_pool(name="const", bufs=1))
        cpool = ctx.enter_context(tc.tile_pool(name="cpool", bufs=4))
        ctpool = ctx.enter_context(tc.tile_pool(name="ctpool", bufs=3))
        qpool = ctx.enter_context(tc.tile_pool(name="qpool", bufs=4))
        spool = ctx.enter_context(tc.tile_pool(name="spool", bufs=4))
        lpool = ctx.enter_context(tc.tile_pool(name="lpool", bufs=3))
        epool = ctx.enter_context(tc.tile_pool(name="epool", bufs=3))
        btpool = ctx.enter_context(tc.tile_pool(name="btpool", bufs=3))
        opool = ctx.enter_context(tc.tile_pool(name="opool", bufs=4))
        ps_tr = ctx.enter_context(tc.tile_pool(name="ps_tr", bufs=2, space="PSUM"))
        ps_mm = ctx.enter_context(tc.tile_pool(name="ps_mm", bufs=2, space="PSUM"))
        ps_sm = ctx.enter_context(tc.tile_pool(name="ps_sm", bufs=2, space="PSUM"))

        # ---- constants (loaded once, reused for all batches) ----
        ident = const.tile([128, 128], BF16, name="ident")
        make_identity(nc, ident)
        identf32 = const.tile([128, 128], F32, name="identf32")
        make_identity(nc, identf32)
        cw_bc = const.tile([Lq, H], F32, name="cw_bc")  # cq_weight bcast over rows
        nc.gpsimd.dma_start(out=cw_bc, in_=bass.AP(tensor=cqw_h, offset=0, ap=[[0, Lq], [1, H]]))
        qw_bc = const.tile([Lq, H], F32, name="qw_bc")  # q_weight bcast over rows
        nc.gpsimd.dma_start(out=qw_bc, in_=bass.AP(tensor=qwgt_h, offset=0, ap=[[0, Lq], [1, H]]))
        cwgt_col = const.tile([128, 4], F32, name="cwgt_col")  # c_weight as 4 chunks
        nc.gpsimd.dma_start(out=cwgt_col, in_=bass.AP(tensor=cwgt_h, offset=0, ap=[[1, 128], [128, 4]]))
        cwgt_hi = const.tile([128, 4], BF16, name="cwgt_hi")
        nc.vector.tensor_copy(out=cwgt_hi, in_=cwgt_col)
        cwgt_res = const.tile([128, 4], F32, name="cwgt_res")
        nc.vector.tensor_sub(cwgt_res, cwgt_col, cwgt_hi)
        cwgt_lo = const.tile([128, 4], BF16, name="cwgt_lo")
        nc.vector.tensor_copy(out=cwgt_lo, in_=cwgt_res)
        bias_bc = const.tile([Lq, 1], F32, name="bias_bc")
        nc.gpsimd.dma_start(out=bias_bc, in_=bass.AP(tensor=bias_h, offset=0, ap=[[0, Lq], [1, 1]]))
        ones_col = const.tile([Lq, 1], BF16, name="ones_col")
        nc.vector.memset(ones_col, 1.0)
        aug_f = const.tile([1, 97], F32, name="aug_f")
        nc.vector.memset(aug_f[:, 0:64], 1.0)
        nc.vector.memset(aug_f[:, 64:97], 0.0)
        aug = const.tile([1, 97], mybir.dt.float32r, name="aug")  # rank-1 s0 add
        nc.vector.tensor_copy(out=aug, in_=aug_f)

        c_tiles = {}
        q_tiles = {}

        def issue_loads(bb):
            c_t = cpool.tile([128, 4, H], F32, name="c_sb")
            nc.sync.dma_start(out=c_t, in_=c_ap[bb].rearrange("(j p) h -> p j h", p=128))
            q_t = qpool.tile([Lq, H], F32, name="q_sb")
            nc.sync.dma_start(out=q_t, in_=q_ap[bb])
            c_tiles[bb] = c_t
            q_tiles[bb] = q_t

        issue_loads(0)
        issue_loads(1)
        for b in range(BPC):
            if b + 2 < BPC:
                issue_loads(b + 2)
            c_sb = c_tiles.pop(b)
            q_sb = q_tiles.pop(b)
            # c passthrough column: only needs the load, issue before compute stores
            nc.sync.dma_start(
                out=out_ap[b, :, 0:512].rearrange("(j p) h -> p j h", p=128),
                in_=c_sb,
            )

            # bf16 copies for matmul operands (c_bf feeds only the M2 matmul)
            c_bf = cpool.tile([128, 4, H], BF16, name="c_bf")
            nc.gpsimd.tensor_copy(out=c_bf[:, 0, :], in_=c_sb[:, 0, :])
            nc.scalar.activation(out=c_bf[:, 1, :], in_=c_sb[:, 1, :], func=copy_f)
            nc.vector.tensor_copy(out=c_bf[:, 2, :], in_=c_sb[:, 2, :])
            nc.gpsimd.tensor_copy(out=c_bf[:, 3, :], in_=c_sb[:, 3, :])
            q_bf = qpool.tile([Lq, H], BF16, name="q_bf")
            nc.gpsimd.tensor_copy(out=q_bf, in_=q_sb)

            # qs = q * cq_weight (bf16 out) ; s1 = (q @ q_weight)
            qs_bf = qpool.tile([Lq, H], BF16, name="qs_bf")
            nc.gpsimd.tensor_mul(qs_bf, q_sb, cw_bc)
            s1_scr = qpool.tile([Lq, H], F32, name="s1_scr")
            s1_raw = spool.tile([Lq, 1], F32, name="s1_raw")
            nc.vector.scalar_tensor_tensor(
                out=s1_scr, in0=q_sb, scalar=1.0, in1=qw_bc,
                op0=MULT, op1=MULT, accum_out=s1_raw,
            )
            s1b = spool.tile([Lq, 1], F32, name="s1b")
            nc.scalar.activation(out=s1b, in_=s1_raw, func=ident_f, bias=bias_bc, scale=1.0)

            # lhsT[f] = [ (qs chunk f)^T | cwgt_hi f | zeros | cwgt_lo f ] -> [128, 97]
            # (hi lands in psum row 64, lo in row 96: engine reads need
            # 32-aligned base partitions)
            lhsT = lpool.tile([128, 4, 97], BF16, name="lhsT")
            for f in range(4):
                pt_q = ps_tr.tile([128, 128], BF16, name="pt_q", tag="tr")
                nc.tensor.transpose(pt_q[:, 0:64], qs_bf[:, ts(f, 128)], ident[0:64, 0:64])
                nc.vector.tensor_copy(out=lhsT[:, f, 0:64], in_=pt_q[:, 0:64])
                nc.vector.memset(lhsT[:, f, 64:97], 0.0)
                nc.vector.tensor_copy(out=lhsT[:, f, 64:65], in_=cwgt_hi[:, f : f + 1])
                nc.vector.tensor_copy(out=lhsT[:, f, 96:97], in_=cwgt_lo[:, f : f + 1])

            # cT[f] = c^T chunk (H rows f*128.., all Lc cols), bf16
            cT = ctpool.tile([128, 4, H], BF16, name="cT")
            for j in range(4):
                for f in range(4):
                    pt_c = ps_tr.tile([128, 128], BF16, name="pt_c", tag="tr")
                    nc.tensor.transpose(pt_c, c_bf[:, j, ts(f, 128)], ident)
                    if (j + f) % 2 == 0:
                        nc.vector.tensor_copy(out=cT[:, f, ts(j, 128)], in_=pt_c)
                    else:
                        nc.scalar.activation(out=cT[:, f, ts(j, 128)], in_=pt_c, func=copy_f)

            # sT accumulation: rows 0..63 = qs@cT, row 64 = s0
            ps_sT = ps_mm.tile([128, 512], F32, name="ps_sT", tag="big1")
            for f in range(4):
                nc.tensor.matmul(
                    ps_sT[0:97, :], lhsT[:, f, :], cT[:, f, :],
                    start=(f == 0), stop=False,
                )
            s0hi = spool.tile([1, H], F32, name="s0hi")
            nc.vector.tensor_copy(out=s0hi, in_=ps_sT[64:65, :])
            s0row = spool.tile([1, H], mybir.dt.float32r, name="s0row")
            nc.vector.tensor_add(s0row, ps_sT[96:97, :], s0hi)
            nc.tensor.matmul(
                ps_sT[0:97, :], aug, s0row,
                start=False, stop=True,
            )

            # E = exp(sT + s1 + bias) in bf16; rowsum (f32) for a2
            E_sb = epool.tile([Lq, H], BF16, name="E_sb")
            rowsum = spool.tile([Lq, 1], F32, name="rowsum")
            nc.scalar.activation(
                out=E_sb, in_=ps_sT[0:64, :], func=exp_f, bias=s1b, scale=1.0,
                accum_out=rowsum,
            )
            ra2 = spool.tile([Lq, 1], F32, name="ra2")
            nc.vector.reciprocal(ra2, rowsum)
            a2T_sb = epool.tile([Lq, H], BF16, name="a2T_sb")
            nc.vector.tensor_scalar_mul(a2T_sb, E_sb, ra2)

            # column sums of E (normalizer of a1), reciprocal per i-tile
            rS = spool.tile([128, 4], F32, name="rS")
            for m in range(4):
                ps_S = ps_sm.tile([128, 1], F32, name="ps_S")
                nc.tensor.matmul(ps_S, E_sb[:, ts(m, 128)], ones_col, start=True, stop=True)
                nc.vector.reciprocal(rS[:, m : m + 1], ps_S)

            # a2 natural layout [Lc, Lq] via PE transposes of a2T
            a2n = btpool.tile([128, 4, Lq], BF16, name="a2n")
            for f in range(4):
                pt_a = ps_tr.tile([128, 128], BF16, name="pt_a", tag="tr")
                nc.tensor.transpose(pt_a[:, 0:64], a2T_sb[:, ts(f, 128)], ident[0:64, 0:64])
                nc.vector.tensor_copy(out=a2n[:, f, :], in_=pt_a[:, 0:64])
            # M2 = a2^T @ c  [Lq, H]  (b = a1 @ M2 afterwards - associativity)
            ps_M2 = ps_mm.tile([128, 512], F32, name="ps_M2", tag="big1")
            for jj in range(4):
                nc.tensor.matmul(
                    ps_M2[0:64, :], a2n[:, jj, :], c_bf[:, jj, :],
                    start=(jj == 0), stop=(jj == 3),
                )
            M2_bf = epool.tile([Lq, H], BF16, name="M2_bf")
            nc.vector.tensor_copy(out=M2_bf, in_=ps_M2[0:64, :])

            for m in range(4):
                stage = opool.tile([128, 3, H], F32, name="stage")
                # a = (E^T chunk @ q) * rS ; ca = c * a
                ps_a = ps_mm.tile([128, 512], F32, name="ps_a", tag="big2")
                nc.tensor.matmul(
                    ps_a, E_sb[:, ts(m, 128)], q_bf,
                    start=True, stop=True,
                )
                nc.scalar.activation(out=stage[:, 0, :], in_=ps_a, func=copy_f, scale=rS[:, m : m + 1])
                nc.vector.tensor_mul(stage[:, 1, :], stage[:, 0, :], c_sb[:, m, :])
                # b = (a1 @ M2) * rS ; cb = c * b
                ps_b = ps_mm.tile([128, 512], F32, name="ps_b", tag="big2")
                nc.tensor.matmul(
                    ps_b, E_sb[:, ts(m, 128)], M2_bf,
                    start=True, stop=True,
                )
                b_sb = opool.tile([128, H], F32, name="b_sb")
                nc.scalar.activation(out=b_sb, in_=ps_b, func=copy_f, scale=rS[:, m : m + 1])
                nc.vector.tensor_mul(stage[:, 2, :], b_sb, c_sb[:, m, :])
                # stores: out = [c | a | c*a | c*b]
                nc.sync.dma_start(out=out_ap[b, ts(m, 128), 512:2048], in_=stage)

    nc.compile()
    return nc


def _numpy_fallback(c, q, c_mask, q_mask, c_weight, q_weight, cq_weight, bias):
    NEG_INF = -1e30
    s0 = c @ c_weight
    s1 = (q @ q_weight).transpose(0, 2, 1)
    s2 = np.einsum("bih,bjh->bij", c * cq_weight, q)
    s = s0 + s1 + s2 + bias

    def softmax(x, mask, axis):
        logits = np.where(mask, x, NEG_INF)
        m = logits.max(axis=axis, keepdims=True)
        e = np.exp(logits - m)
        return e / e.sum(axis=axis, keepdims=True)

    a1 = softmax(s, q_mask[:, None, :], 2)
    a2 = softmax(s, c_mask[:, :, None], 1)
    a = np.einsum("bij,bjh->bih", a1, q)
    bb = np.einsum("bik,bjk->bij", a1, a2)
    bb = np.einsum("bij,bjh->bih", bb, c)
    return np.concatenate([c, a, c * a, c * bb], axis=2).astype(np.float32)


def kernel(c, q, c_mask, q_mask, c_weight, q_weight, cq_weight, bias, **_):
    c = np.asarray(c, dtype=np.float32)
    q = np.asarray(q, dtype=np.float32)
    if not (np.all(c_mask) and np.all(q_mask)):
        # masks are all-ones per the problem spec; keep a correct fallback
        return _numpy_fallback(
            c, q, np.asarray(c_mask), np.asarray(q_mask),
            np.asarray(c_weight, np.float32), np.asarray(q_weight, np.float32),
            np.asarray(cq_weight, np.float32), np.asarray(bias, np.float32),
        )

    if "nc" not in _CACHE:
        _CACHE["nc"] = _build_program()
    nc = _CACHE["nc"]

    cqw = np.ascontiguousarray(np.asarray(cq_weight, np.float32).reshape(H))
    cwgt = np.ascontiguousarray(np.asarray(c_weight, np.float32).reshape(H))
    qwgt = np.ascontiguousarray(np.asarray(q_weight, np.float32).reshape(H))
    bias_a = np.ascontiguousarray(np.asarray(bias, np.float32).reshape(1))

    in_maps = []
    for k in range(N_CORES):
        in_maps.append(
            {
                "c": np.ascontiguousarray(c[k * BPC : (k + 1) * BPC]),
                "q": np.ascontiguousarray(q[k * BPC : (k + 1) * BPC]),
                "cqw": cqw,
                "cwgt": cwgt,
                "qwgt": qwgt,
                "bias": bias_a,
            }
        )
    res = run_bass_kernel_spmd(nc, in_maps, core_ids=list(range(N_CORES)))
    return np.concatenate([res.results[k]["out"] for k in range(N_CORES)], axis=0)


# revision 18
# speedup vs baseline: 1.0865x; 1.0865x over previous
"""BiDAF attention kernel for 8 Trainium2 NeuronCores.

Data-parallel over batch (B=32 -> 4 per core). Per batch, on-chip:
  sT[j,i] = (q*cqw) @ c^T + s0[i] + s1[j] + bias   (bf16 matmuls, fp32 accum;
  s0 comes free as row 64 of the same matmul, then a rank-1 augmentation
  matmul broadcasts it across rows)
  E = exp(sT)  (one exp serves both softmaxes; s1+bias fused via act bias)
  a2T = E / rowsum(E);  a1 normalization deferred: 1/colsum(E) applied to
  output rows of downstream matmuls.
  a = a1 @ q; binner = a1 @ a2^T; b = binner @ c; out = [c, a, c*a, c*b]
"""

import sys

if "/opt/trn_rl_repo" not in sys.path:
    sys.path.insert(0, "/opt/trn_rl_repo")

from contextlib import ExitStack

import numpy as np

import concourse.bacc as bacc
import concourse.bass as bass
import concourse.mybir as mybir
from concourse.bass import ts
from concourse.bass_utils import run_bass_kernel_spmd
from concourse.masks import make_identity
from concourse.tile import TileContext

N_CORES = 8
B, Lc, Lq, H = 32, 512, 64, 512
BPC = B // N_CORES  # batches per core
F32 = mybir.dt.float32
BF16 = mybir.dt.bfloat16
MULT = mybir.AluOpType.mult

_CACHE = {}


def _build_program():
    nc = bacc.Bacc("TRN2", target_bir_lowering=False, debug=False, num_devices=N_CORES)
    c_h = nc.dram_tensor("c", [BPC, Lc, H], F32, kind="ExternalInput")
    q_h = nc.dram_tensor("q", [BPC, Lq, H], F32, kind="ExternalInput")
    cqw_h = nc.dram_tensor("cqw", [H], F32, kind="ExternalInput")
    cwgt_h = nc.dram_tensor("cwgt", [H], F32, kind="ExternalInput")
    qwgt_h = nc.dram_tensor("qwgt", [H], F32, kind="ExternalInput")
    bias_h = nc.dram_tensor("bias", [1], F32, kind="ExternalInput")
    out_h = nc.dram_tensor("out", [BPC, Lc, 4 * H], F32, kind="ExternalOutput")

    c_ap = c_h.ap()
    q_ap = q_h.ap()
    out_ap = out_h.ap()

    exp_f = mybir.ActivationFunctionType.Exp
    ident_f = mybir.ActivationFunctionType.Identity
    copy_f = mybir.ActivationFunctionType.Copy

    with TileContext(nc) as tc, ExitStack() as ctx:
        const = ctx.enter_context(tc.tile_pool(name="const", bufs=1))
        cpool = ctx.enter_context(tc.tile_pool(name="cpool", bufs=4))
        ctpool = ctx.enter_context(tc.tile_pool(name="ctpool", bufs=3))
        qpool = ctx.enter_context(tc.tile_pool(name="qpool", bufs=4))
        spool = ctx.enter_context(tc.tile_pool(name="spool", bufs=4))
        lpool = ctx.enter_context(tc.tile_pool(name="lpool", bufs=3))
        epool = ctx.enter_context(tc.tile_pool(name="epool", bufs=3))
        btpool = ctx.enter_context(tc.tile_pool(name="btpool", bufs=3))
        opool = ctx.enter_context(tc.tile_pool(name="opool", bufs=4))
        ps_tr = ctx.enter_context(tc.tile_pool(name="ps_tr", bufs=2, space="PSUM"))
        ps_mm = ctx.enter_context(tc.tile_pool(name="ps_mm", bufs=2, space="PSUM"))
        ps_sm = ctx.enter_context(tc.tile_pool(name="ps_sm", bufs=2, space="PSUM"))

        # ---- constants (loaded once, reused for all batches) ----
        ident = const.tile([128, 128], BF16, name="ident")
        make_identity(nc, ident)
        identf32 = const.tile([128, 128], F32, name="identf32")
        make_identity(nc, identf32)
        cw_bc = const.tile([Lq, H], F32, name="cw_bc")  # cq_weight bcast over rows
        nc.gpsimd.dma_start(out=cw_bc, in_=bass.AP(tensor=cqw_h, offset=0, ap=[[0, Lq], [1, H]]))
        qw_bc = const.tile([Lq, H], F32, name="qw_bc")  # q_weight bcast over rows
        nc.gpsimd.dma_start(out=qw_bc, in_=bass.AP(tensor=qwgt_h, offset=0, ap=[[0, Lq], [1, H]]))
        cwgt_col = const.tile([128, 4], F32, name="cwgt_col")  # c_weight as 4 chunks
        nc.gpsimd.dma_start(out=cwgt_col, in_=bass.AP(tensor=cwgt_h, offset=0, ap=[[1, 128], [128, 4]]))
        cwgt_hi = const.tile([128, 4], BF16, name="cwgt_hi")
        nc.vector.tensor_copy(out=cwgt_hi, in_=cwgt_col)
        cwgt_res = const.tile([128, 4], F32, name="cwgt_res")
        nc.vector.tensor_sub(cwgt_res, cwgt_col, cwgt_hi)
        cwgt_lo = const.tile([128, 4], BF16, name="cwgt_lo")
        nc.vector.tensor_copy(out=cwgt_lo, in_=cwgt_res)
        bias_bc = const.tile([Lq, 1], F32, name="bias_bc")
        nc.gpsimd.dma_start(out=bias_bc, in_=bass.AP(tensor=bias_h, offset=0, ap=[[0, Lq], [1, 1]]))
        ones_col = const.tile([Lq, 1], BF16, name="ones_col")
        nc.vector.memset(ones_col, 1.0)
        aug_f = const.tile([1, 97], F32, name="aug_f")
        nc.vector.memset(aug_f[:, 0:64], 1.0)
        nc.vector.memset(aug_f[:, 64:97], 0.0)
        aug = const.tile([1, 97], mybir.dt.float32r, name="aug")  # rank-1 s0 add
        nc.vector.tensor_copy(out=aug, in_=aug_f)

        c_tiles = {}
        q_tiles = {}

        def issue_loads(bb):
            c_t = cpool.tile([128, 4, H], F32, name="c_sb")
            nc.sync.dma_start(out=c_t, in_=c_ap[bb].rearrange("(j p) h -> p j h", p=128))
            q_t = qpool.tile([Lq, H], F32, name="q_sb")
            nc.sync.dma_start(out=q_t, in_=q_ap[bb])
            c_tiles[bb] = c_t
            q_tiles[bb] = q_t

        issue_loads(0)
        issue_loads(1)
        for b in range(BPC):
            if b + 2 < BPC:
                issue_loads(b + 2)
            c_sb = c_tiles.pop(b)
            q_sb = q_tiles.pop(b)
            # c passthrough column: only needs the load, issue before compute stores
            nc.sync.dma_start(
                out=out_ap[b, :, 0:512].rearrange("(j p) h -> p j h", p=128),
                in_=c_sb,
            )

            # bf16 copies for matmul operands (c_bf feeds only the M2 matmul)
            c_bf = cpool.tile([128, 4, H], BF16, name="c_bf")
            nc.scalar.activation(out=c_bf[:, 0, :], in_=c_sb[:, 0, :], func=copy_f)
            nc.vector.tensor_copy(out=c_bf[:, 1, :], in_=c_sb[:, 1, :])
            nc.scalar.activation(out=c_bf[:, 2, :], in_=c_sb[:, 2, :], func=copy_f)
            nc.vector.tensor_copy(out=c_bf[:, 3, :], in_=c_sb[:, 3, :])
            q_bf = qpool.tile([Lq, H], BF16, name="q_bf")
            nc.gpsimd.tensor_copy(out=q_bf, in_=q_sb)

            # qs = q * cq_weight (bf16 out) ; s1 = (q @ q_weight)
            qs_bf = qpool.tile([Lq, H], BF16, name="qs_bf")
            nc.vector.tensor_mul(qs_bf, q_sb, cw_bc)
            s1_scr = qpool.tile([Lq, H], F32, name="s1_scr")
            s1_raw = spool.tile([Lq, 1], F32, name="s1_raw")
            nc.gpsimd.tensor_mul(s1_scr, q_sb, qw_bc)
            nc.vector.tensor_reduce(
                out=s1_raw, in_=s1_scr, axis=mybir.AxisListType.X,
                op=mybir.AluOpType.add,
            )
            s1b = spool.tile([Lq, 1], F32, name="s1b")
            nc.scalar.activation(out=s1b, in_=s1_raw, func=ident_f, bias=bias_bc, scale=1.0)

            # lhsT[f] = [ (qs chunk f)^T | cwgt_hi f | zeros | cwgt_lo f ] -> [128, 97]
            # (hi lands in psum row 64, lo in row 96: engine reads need
            # 32-aligned base partitions)
            lhsT = lpool.tile([128, 4, 97], BF16, name="lhsT")
            for f in range(4):
                pt_q = ps_tr.tile([128, 128], BF16, name="pt_q", tag="tr")
                nc.tensor.transpose(pt_q[:, 0:64], qs_bf[:, ts(f, 128)], ident[0:64, 0:64])
                nc.vector.tensor_copy(out=lhsT[:, f, 0:64], in_=pt_q[:, 0:64])
                nc.vector.memset(lhsT[:, f, 64:97], 0.0)
                nc.vector.tensor_copy(out=lhsT[:, f, 64:65], in_=cwgt_hi[:, f : f + 1])
                nc.vector.tensor_copy(out=lhsT[:, f, 96:97], in_=cwgt_lo[:, f : f + 1])

            # cT[f] = c^T chunk (H rows f*128.., all Lc cols), bf16
            cT = ctpool.tile([128, 4, H], BF16, name="cT")
            for j in range(4):
                for f in range(4):
                    pt_c = ps_tr.tile([128, 128], BF16, name="pt_c", tag="tr")
                    nc.tensor.transpose(pt_c, c_bf[:, j, ts(f, 128)], ident)
                    if (j + f) % 2 == 0:
                        nc.vector.tensor_copy(out=cT[:, f, ts(j, 128)], in_=pt_c)
                    else:
                        nc.scalar.activation(out=cT[:, f, ts(j, 128)], in_=pt_c, func=copy_f)

            # sT accumulation: rows 0..63 = qs@cT, row 64 = s0
            ps_sT = ps_mm.tile([128, 512], F32, name="ps_sT", tag="big1")
            for f in range(4):
                nc.tensor.matmul(
                    ps_sT[0:97, :], lhsT[:, f, :], cT[:, f, :],
                    start=(f == 0), stop=False,
                )
            s0hi = spool.tile([1, H], F32, name="s0hi")
            nc.vector.tensor_copy(out=s0hi, in_=ps_sT[64:65, :])
            s0row = spool.tile([1, H], mybir.dt.float32r, name="s0row")
            nc.vector.tensor_add(s0row, ps_sT[96:97, :], s0hi)
            nc.tensor.matmul(
                ps_sT[0:97, :], aug, s0row,
                start=False, stop=True,
            )

            # E = exp(sT + s1 + bias) in bf16; rowsum (f32) for a2
            E_sb = epool.tile([Lq, H], BF16, name="E_sb")
            rowsum = spool.tile([Lq, 1], F32, name="rowsum")
            nc.scalar.activation(
                out=E_sb, in_=ps_sT[0:64, :], func=exp_f, bias=s1b, scale=1.0,
                accum_out=rowsum,
            )
            ra2 = spool.tile([Lq, 1], F32, name="ra2")
            nc.vector.reciprocal(ra2, rowsum)
            a2T_sb = epool.tile([Lq, H], BF16, name="a2T_sb")
            nc.vector.tensor_scalar_mul(a2T_sb, E_sb, ra2)

            # column sums of E (normalizer of a1), reciprocal per i-tile
            rS = spool.tile([128, 4], F32, name="rS")
            for m in range(4):
                ps_S = ps_sm.tile([128, 1], F32, name="ps_S")
                nc.tensor.matmul(ps_S, E_sb[:, ts(m, 128)], ones_col, start=True, stop=True)
                nc.vector.reciprocal(rS[:, m : m + 1], ps_S)

            # a2 natural layout [Lc, Lq] via PE transposes of a2T
            a2n = btpool.tile([128, 4, Lq], BF16, name="a2n")
            for f in range(4):
                pt_a = ps_tr.tile([128, 128], BF16, name="pt_a", tag="tr")
                nc.tensor.transpose(pt_a[:, 0:64], a2T_sb[:, ts(f, 128)], ident[0:64, 0:64])
                nc.vector.tensor_copy(out=a2n[:, f, :], in_=pt_a[:, 0:64])
            # M2 = a2^T @ c  [Lq, H]  (b = a1 @ M2 afterwards - associativity)
            ps_M2 = ps_mm.tile([128, 512], F32, name="ps_M2", tag="big1")
            for jj in range(4):
                nc.tensor.matmul(
                    ps_M2[0:64, :], a2n[:, jj, :], c_bf[:, jj, :],
                    start=(jj == 0), stop=(jj == 3),
                )
            M2_bf = epool.tile([Lq, H], BF16, name="M2_bf")
            nc.vector.tensor_copy(out=M2_bf, in_=ps_M2[0:64, :])

            for m in range(4):
                stage = opool.tile([128, 3, H], F32, name="stage")
                # a = (E^T chunk @ q) * rS ; ca = c * a
                ps_a = ps_mm.tile([128, 512], F32, name="ps_a", tag="big2")
                nc.tensor.matmul(
                    ps_a, E_sb[:, ts(m, 128)], q_bf,
                    start=True, stop=True,
                )
                nc.scalar.activation(out=stage[:, 0, :], in_=ps_a, func=copy_f, scale=rS[:, m : m + 1])
                nc.vector.tensor_mul(stage[:, 1, :], stage[:, 0, :], c_sb[:, m, :])
                # b = (a1 @ M2) * rS ; cb = c * b
                ps_b = ps_mm.tile([128, 512], F32, name="ps_b", tag="big2")
                nc.tensor.matmul(
                    ps_b, E_sb[:, ts(m, 128)], M2_bf,
                    start=True, stop=True,
                )
                nc.vector.scalar_tensor_tensor(
                    out=stage[:, 2, :], in0=ps_b, scalar=rS[:, m : m + 1], in1=c_sb[:, m, :],
                    op0=MULT, op1=MULT,
                )
                # stores: out = [c | a | c*a | c*b]
                nc.sync.dma_start(out=out_ap[b, ts(m, 128), 512:2048], in_=stage)

    nc.compile()
    return nc


def _numpy_fallback(c, q, c_mask, q_mask, c_weight, q_weight, cq_weight, bias):
    NEG_INF = -1e30
    s0 = c @ c_weight
    s1 = (q @ q_weight).transpose(0, 2, 1)
    s2 = np.einsum("bih,bjh->bij", c * cq_weight, q)
    s = s0 + s1 + s2 + bias

    def softmax(x, mask, axis):
        logits = np.where(mask, x, NEG_INF)
        m = logits.max(axis=axis, keepdims=True)
        e = np.exp(logits - m)
        return e / e.sum(axis=axis, keepdims=True)

    a1 = softmax(s, q_mask[:, None, :], 2)
    a2 = softmax(s, c_mask[:, :, None], 1)
    a = np.einsum("bij,bjh->bih", a1, q)
    bb = np.einsum("bik,bjk->bij", a1, a2)
    bb = np.einsum("bij,bjh->bih", bb, c)
    return np.concatenate([c, a, c * a, c * bb], axis=2).astype(np.float32)


def kernel(c, q, c_mask, q_mask, c_weight, q_weight, cq_weight, bias, **_):
    c = np.asarray(c, dtype=np.float32)
    q = np.asarray(q, dtype=np.float32)
    if not (np.all(c_mask) and np.all(q_mask)):
        # masks are all-ones per the problem spec; keep a correct fallback
        return _numpy_fallback(
            c, q, np.asarray(c_mask), np.asarray(q_mask),
            np.asarray(c_weight, np.float32), np.asarray(q_weight, np.float32),
            np.asarray(cq_weight, np.float32), np.asarray(bias, np.float32),
        )

    if "nc" not in _CACHE:
        _CACHE["nc"] = _build_program()
    nc = _CACHE["nc"]

    cqw = np.ascontiguousarray(np.asarray(cq_weight, np.float32).reshape(H))
    cwgt = np.ascontiguousarray(np.asarray(c_weight, np.float32).reshape(H))
    qwgt = np.ascontiguousarray(np.asarray(q_weight, np.float32).reshape(H))
    bias_a = np.ascontiguousarray(np.asarray(bias, np.float32).reshape(1))

    in_maps = []
    for k in range(N_CORES):
        in_maps.append(
            {
                "c": np.ascontiguousarray(c[k * BPC : (k + 1) * BPC]),
                "q": np.ascontiguousarray(q[k * BPC : (k + 1) * BPC]),
                "cqw": cqw,
                "cwgt": cwgt,
                "qwgt": qwgt,
                "bias": bias_a,
            }
        )
    res = run_bass_kernel_spmd(nc, in_maps, core_ids=list(range(N_CORES)))
    return np.concatenate([res.results[k]["out"] for k in range(N_CORES)], axis=0)


# revision 20
# speedup vs baseline: 1.2135x; 1.1169x over previous
"""BiDAF attention kernel for 8 Trainium2 NeuronCores.

Data-parallel over batch (B=32 -> 4 per core). Per batch, on-chip:
  sT[j,i] = (q*cqw) @ c^T + s0[i] + s1[j] + bias   (bf16 matmuls, fp32 accum;
  s0 comes free as row 64 of the same matmul, then a rank-1 augmentation
  matmul broadcasts it across rows)
  E = exp(sT)  (one exp serves both softmaxes; s1+bias fused via act bias)
  a2T = E / rowsum(E);  a1 normalization deferred: 1/colsum(E) applied to
  output rows of downstream matmuls.
  a = a1 @ q; binner = a1 @ a2^T; b = binner @ c; out = [c, a, c*a, c*b]
"""

import sys

if "/opt/trn_rl_repo" not in sys.path:
    sys.path.insert(0, "/opt/trn_rl_repo")

from contextlib import ExitStack

import numpy as np

import concourse.bacc as bacc
import concourse.bass as bass
import concourse.mybir as mybir
from concourse.bass import ts
from concourse.bass_utils import run_bass_kernel_spmd
from concourse.masks import make_identity
from concourse.tile import TileContext

N_CORES = 8
B, Lc, Lq, H = 32, 512, 64, 512
BPC = B // N_CORES  # batches per core
F32 = mybir.dt.float32
BF16 = mybir.dt.bfloat16
MULT = mybir.AluOpType.mult

_CACHE = {}


def _build_program():
    nc = bacc.Bacc("TRN2", target_bir_lowering=False, debug=False, num_devices=N_CORES)
    c_h = nc.dram_tensor("c", [BPC, Lc, H], F32, kind="ExternalInput")
    q_h = nc.dram_tensor("q", [BPC, Lq, H], F32, kind="ExternalInput")
    cqw_h = nc.dram_tensor("cqw", [H], F32, kind="ExternalInput")
    cwgt_h = nc.dram_tensor("cwgt", [H], F32, kind="ExternalInput")
    qwgt_h = nc.dram_tensor("qwgt", [H], F32, kind="ExternalInput")
    bias_h = nc.dram_tensor("bias", [1], F32, kind="ExternalInput")
    out_h = nc.dram_tensor("out", [BPC, Lc, 4 * H], F32, kind="ExternalOutput")

    c_ap = c_h.ap()
    q_ap = q_h.ap()
    out_ap = out_h.ap()

    exp_f = mybir.ActivationFunctionType.Exp
    ident_f = mybir.ActivationFunctionType.Identity
    copy_f = mybir.ActivationFunctionType.Copy

    with TileContext(nc) as tc, ExitStack() as ctx:
        const = ctx.enter_context(tc.tile_pool(name="const", bufs=1))
        cpool = ctx.enter_context(tc.tile_pool(name="cpool", bufs=4))
        ctpool = ctx.enter_context(tc.tile_pool(name="ctpool", bufs=3))
        qpool = ctx.enter_context(tc.tile_pool(name="qpool", bufs=4))
        spool = ctx.enter_context(tc.tile_pool(name="spool", bufs=4))
        lpool = ctx.enter_context(tc.tile_pool(name="lpool", bufs=3))
        epool = ctx.enter_context(tc.tile_pool(name="epool", bufs=3))
        btpool = ctx.enter_context(tc.tile_pool(name="btpool", bufs=3))
        opool = ctx.enter_context(tc.tile_pool(name="opool", bufs=4))
        ps_tr = ctx.enter_context(tc.tile_pool(name="ps_tr", bufs=2, space="PSUM"))
        ps_mm = ctx.enter_context(tc.tile_pool(name="ps_mm", bufs=2, space="PSUM"))
        ps_sm = ctx.enter_context(tc.tile_pool(name="ps_sm", bufs=1, space="PSUM"))

        # ---- constants (loaded once, reused for all batches) ----
        ident = const.tile([128, 128], BF16, name="ident")
        make_identity(nc, ident)
        identf32 = const.tile([128, 128], F32, name="identf32")
        make_identity(nc, identf32)
        cw_bc = const.tile([Lq, H], F32, name="cw_bc")  # cq_weight bcast over rows
        nc.gpsimd.dma_start(out=cw_bc, in_=bass.AP(tensor=cqw_h, offset=0, ap=[[0, Lq], [1, H]]))
        qw_bc = const.tile([Lq, H], F32, name="qw_bc")  # q_weight bcast over rows
        nc.gpsimd.dma_start(out=qw_bc, in_=bass.AP(tensor=qwgt_h, offset=0, ap=[[0, Lq], [1, H]]))
        cwgt_col = const.tile([128, 4], F32, name="cwgt_col")  # c_weight as 4 chunks
        nc.gpsimd.dma_start(out=cwgt_col, in_=bass.AP(tensor=cwgt_h, offset=0, ap=[[1, 128], [128, 4]]))
        cwgt_hi = const.tile([128, 4], BF16, name="cwgt_hi")
        nc.vector.tensor_copy(out=cwgt_hi, in_=cwgt_col)
        cwgt_res = const.tile([128, 4], F32, name="cwgt_res")
        nc.vector.tensor_sub(cwgt_res, cwgt_col, cwgt_hi)
        cwgt_lo = const.tile([128, 4], BF16, name="cwgt_lo")
        nc.vector.tensor_copy(out=cwgt_lo, in_=cwgt_res)
        bias_bc = const.tile([Lq, 1], F32, name="bias_bc")
        nc.gpsimd.dma_start(out=bias_bc, in_=bass.AP(tensor=bias_h, offset=0, ap=[[0, Lq], [1, 1]]))
        ones_col = const.tile([Lq, 1], BF16, name="ones_col")
        nc.vector.memset(ones_col, 1.0)
        aug_f = const.tile([1, 97], F32, name="aug_f")
        nc.vector.memset(aug_f[:, 0:64], 1.0)
        nc.vector.memset(aug_f[:, 64:97], 0.0)
        aug = const.tile([1, 97], mybir.dt.float32r, name="aug")  # rank-1 s0 add
        nc.vector.tensor_copy(out=aug, in_=aug_f)

        c_tiles = {}
        q_tiles = {}
        S = [dict() for _ in range(BPC)]  # per-batch tile state

        def issue_loads(bb):
            c_t = cpool.tile([128, 4, H], F32, name="c_sb")
            nc.sync.dma_start(out=c_t, in_=c_ap[bb].rearrange("(j p) h -> p j h", p=128))
            q_t = qpool.tile([Lq, H], F32, name="q_sb")
            nc.sync.dma_start(out=q_t, in_=q_ap[bb])
            c_tiles[bb] = c_t
            q_tiles[bb] = q_t

        def stage_A(b):
            """loads -> bf16 casts -> transposes -> sT matmuls -> exp"""
            c_sb = c_tiles[b]
            q_sb = q_tiles[b]
            # c passthrough column: only needs the load
            nc.sync.dma_start(
                out=out_ap[b, :, 0:512].rearrange("(j p) h -> p j h", p=128),
                in_=c_sb,
            )

            c_bf = cpool.tile([128, 4, H], BF16, name="c_bf")
            nc.scalar.activation(out=c_bf[:, 0, :], in_=c_sb[:, 0, :], func=copy_f)
            nc.vector.tensor_copy(out=c_bf[:, 1, :], in_=c_sb[:, 1, :])
            nc.scalar.activation(out=c_bf[:, 2, :], in_=c_sb[:, 2, :], func=copy_f)
            nc.vector.tensor_copy(out=c_bf[:, 3, :], in_=c_sb[:, 3, :])
            q_bf = qpool.tile([Lq, H], BF16, name="q_bf")
            nc.gpsimd.tensor_copy(out=q_bf, in_=q_sb)

            # qs = q * cq_weight (bf16 out) ; s1 = (q @ q_weight)
            qs_bf = qpool.tile([Lq, H], BF16, name="qs_bf")
            nc.vector.tensor_mul(qs_bf, q_sb, cw_bc)
            s1_scr = qpool.tile([Lq, H], F32, name="s1_scr")
            s1_raw = spool.tile([Lq, 1], F32, name="s1_raw")
            nc.gpsimd.tensor_mul(s1_scr, q_sb, qw_bc)
            nc.vector.tensor_reduce(
                out=s1_raw, in_=s1_scr, axis=mybir.AxisListType.X,
                op=mybir.AluOpType.add,
            )
            s1b = spool.tile([Lq, 1], F32, name="s1b")
            nc.scalar.activation(out=s1b, in_=s1_raw, func=ident_f, bias=bias_bc, scale=1.0)

            # lhsT[f] = [ (qs chunk f)^T | cwgt_hi f | zeros | cwgt_lo f ] -> [128, 97]
            # (hi lands in psum row 64, lo in row 96: engine reads need
            # 32-aligned base partitions)
            lhsT = lpool.tile([128, 4, 97], BF16, name="lhsT")
            for f in range(4):
                pt_q = ps_tr.tile([128, 128], BF16, name="pt_q", tag="tr")
                nc.tensor.transpose(pt_q[:, 0:64], qs_bf[:, ts(f, 128)], ident[0:64, 0:64])
                nc.vector.tensor_copy(out=lhsT[:, f, 0:64], in_=pt_q[:, 0:64])
                nc.vector.memset(lhsT[:, f, 64:97], 0.0)
                nc.vector.tensor_copy(out=lhsT[:, f, 64:65], in_=cwgt_hi[:, f : f + 1])
                nc.vector.tensor_copy(out=lhsT[:, f, 96:97], in_=cwgt_lo[:, f : f + 1])

            # cT[f] = c^T chunk (H rows f*128.., all Lc cols), bf16
            cT = ctpool.tile([128, 4, H], BF16, name="cT")
            for j in range(4):
                for f in range(4):
                    pt_c = ps_tr.tile([128, 128], BF16, name="pt_c", tag="tr")
                    nc.tensor.transpose(pt_c, c_bf[:, j, ts(f, 128)], ident)
                    if (j + f) % 2 == 0:
                        nc.vector.tensor_copy(out=cT[:, f, ts(j, 128)], in_=pt_c)
                    else:
                        nc.scalar.activation(out=cT[:, f, ts(j, 128)], in_=pt_c, func=copy_f)

            # sT accumulation: rows 0..63 = qs@cT, row 64/96 = s0 hi/lo parts
            ps_sT = ps_mm.tile([128, 512], F32, name="ps_sT", tag="big1", bufs=3)
            for f in range(4):
                nc.tensor.matmul(
                    ps_sT[0:97, :], lhsT[:, f, :], cT[:, f, :],
                    start=(f == 0), stop=False,
                )
            s0hi = spool.tile([1, H], F32, name="s0hi")
            nc.vector.tensor_copy(out=s0hi, in_=ps_sT[64:65, :])
            s0row = spool.tile([1, H], mybir.dt.float32r, name="s0row")
            nc.vector.tensor_add(s0row, ps_sT[96:97, :], s0hi)
            nc.tensor.matmul(
                ps_sT[0:97, :], aug, s0row,
                start=False, stop=True,
            )

            # E = exp(sT + s1 + bias) in bf16; rowsum (f32) for a2
            E_sb = epool.tile([Lq, H], BF16, name="E_sb")
            rowsum = spool.tile([Lq, 1], F32, name="rowsum")
            nc.scalar.activation(
                out=E_sb, in_=ps_sT[0:64, :], func=exp_f, bias=s1b, scale=1.0,
                accum_out=rowsum,
            )
            S[b].update(c_sb=c_sb, q_bf=q_bf, c_bf=c_bf, E_sb=E_sb, rowsum=rowsum)

        def stage_B(b):
            """a2 softmax -> a2 transpose -> M2 = a2^T @ c ; colsum normalizers"""
            c_bf = S[b]["c_bf"]
            E_sb = S[b]["E_sb"]
            ra2 = spool.tile([Lq, 1], F32, name="ra2")
            nc.vector.reciprocal(ra2, S[b]["rowsum"])
            a2T_sb = epool.tile([Lq, H], BF16, name="a2T_sb")
            nc.vector.tensor_scalar_mul(a2T_sb, E_sb, ra2)

            # column sums of E (normalizer of a1), reciprocal per i-tile
            rS = spool.tile([128, 4], F32, name="rS")
            for m in range(4):
                ps_S = ps_sm.tile([128, 1], F32, name="ps_S")
                nc.tensor.matmul(ps_S, E_sb[:, ts(m, 128)], ones_col, start=True, stop=True)
                nc.vector.reciprocal(rS[:, m : m + 1], ps_S)

            # a2 natural layout [Lc, Lq] via PE transposes of a2T
            a2n = btpool.tile([128, 4, Lq], BF16, name="a2n")
            for f in range(4):
                pt_a = ps_tr.tile([128, 128], BF16, name="pt_a", tag="tr")
                nc.tensor.transpose(pt_a[:, 0:64], a2T_sb[:, ts(f, 128)], ident[0:64, 0:64])
                nc.vector.tensor_copy(out=a2n[:, f, :], in_=pt_a[:, 0:64])
            # M2 = a2^T @ c  [Lq, H]  (b = a1 @ M2 afterwards - associativity)
            ps_M2 = ps_mm.tile([128, 512], F32, name="ps_M2", tag="big1", bufs=3)
            for jj in range(4):
                nc.tensor.matmul(
                    ps_M2[0:64, :], a2n[:, jj, :], c_bf[:, jj, :],
                    start=(jj == 0), stop=(jj == 3),
                )
            M2_bf = epool.tile([Lq, H], BF16, name="M2_bf")
            nc.vector.tensor_copy(out=M2_bf, in_=ps_M2[0:64, :])
            S[b].update(rS=rS, M2_bf=M2_bf)

        def stage_C(b):
            """per i-tile: a / ca / b / cb + stores"""
            c_sb = S[b]["c_sb"]
            q_bf = S[b]["q_bf"]
            E_sb = S[b]["E_sb"]
            rS = S[b]["rS"]
            M2_bf = S[b]["M2_bf"]
            for m in range(4):
                stage = opool.tile([128, 3, H], F32, name="stage")
                # a = (E^T chunk @ q) * rS ; ca = c * a
                ps_a = ps_mm.tile([128, 512], F32, name="ps_a", tag="big2")
                nc.tensor.matmul(
                    ps_a, E_sb[:, ts(m, 128)], q_bf,
                    start=True, stop=True,
                )
                nc.scalar.activation(out=stage[:, 0, :], in_=ps_a, func=copy_f, scale=rS[:, m : m + 1])
                nc.vector.tensor_mul(stage[:, 1, :], stage[:, 0, :], c_sb[:, m, :])
                # b = (a1 @ M2) * rS ; cb = c * b
                ps_b = ps_mm.tile([128, 512], F32, name="ps_b", tag="big2")
                nc.tensor.matmul(
                    ps_b, E_sb[:, ts(m, 128)], M2_bf,
                    start=True, stop=True,
                )
                nc.vector.scalar_tensor_tensor(
                    out=stage[:, 2, :], in0=ps_b, scalar=rS[:, m : m + 1], in1=c_sb[:, m, :],
                    op0=MULT, op1=MULT,
                )
                # stores: out = [c | a | c*a | c*b]
                nc.sync.dma_start(out=out_ap[b, ts(m, 128), 512:2048], in_=stage)
            S[b].clear()

        # software-pipelined emission: A(b+2) | B(b+1) | C(b)
        issue_loads(0)
        issue_loads(1)
        stage_A(0)
        issue_loads(2)
        stage_A(1)
        stage_B(0)
        issue_loads(3)
        stage_A(2)
        stage_B(1)
        stage_C(0)
        stage_A(3)
        stage_B(2)
        stage_C(1)
        stage_B(3)
        stage_C(2)
        stage_C(3)

    nc.compile()
    return nc


def _numpy_fallback(c, q, c_mask, q_mask, c_weight, q_weight, cq_weight, bias):
    NEG_INF = -1e30
    s0 = c @ c_weight
    s1 = (q @ q_weight).transpose(0, 2, 1)
    s2 = np.einsum("bih,bjh->bij", c * cq_weight, q)
    s = s0 + s1 + s2 + bias

    def softmax(x, mask, axis):
        logits = np.where(mask, x, NEG_INF)
        m = logits.max(axis=axis, keepdims=True)
        e = np.exp(logits - m)
        return e / e.sum(axis=axis, keepdims=True)

    a1 = softmax(s, q_mask[:, None, :], 2)
    a2 = softmax(s, c_mask[:, :, None], 1)
    a = np.einsum("bij,bjh->bih", a1, q)
    bb = np.einsum("bik,bjk->bij", a1, a2)
    bb = np.einsum("bij,bjh->bih", bb, c)
    return np.concatenate([c, a, c * a, c * bb], axis=2).astype(np.float32)


def kernel(c, q, c_mask, q_mask, c_weight, q_weight, cq_weight, bias, **_):
    c = np.asarray(c, dtype=np.float32)
    q = np.asarray(q, dtype=np.float32)
    if not (np.all(c_mask) and np.all(q_mask)):
        # masks are all-ones per the problem spec; keep a correct fallback
        return _numpy_fallback(
            c, q, np.asarray(c_mask), np.asarray(q_mask),
            np.asarray(c_weight, np.float32), np.asarray(q_weight, np.float32),
            np.asarray(cq_weight, np.float32), np.asarray(bias, np.float32),
        )

    if "nc" not in _CACHE:
        _CACHE["nc"] = _build_program()
    nc = _CACHE["nc"]

    cqw = np.ascontiguousarray(np.asarray(cq_weight, np.float32).reshape(H))
    cwgt = np.ascontiguousarray(np.asarray(c_weight, np.float32).reshape(H))
    qwgt = np.ascontiguousarray(np.asarray(q_weight, np.float32).reshape(H))
    bias_a = np.ascontiguousarray(np.asarray(bias, np.float32).reshape(1))

    in_maps = []
    for k in range(N_CORES):
        in_maps.append(
            {
                "c": np.ascontiguousarray(c[k * BPC : (k + 1) * BPC]),
                "q": np.ascontiguousarray(q[k * BPC : (k + 1) * BPC]),
                "cqw": cqw,
                "cwgt": cwgt,
                "qwgt": qwgt,
                "bias": bias_a,
            }
        )
    res = run_bass_kernel_spmd(nc, in_maps, core_ids=list(range(N_CORES)))
    return np.concatenate([res.results[k]["out"] for k in range(N_CORES)], axis=0)
